# revision 26
# baseline (speedup 1.0000x reference)
"""Multi-head causal self-attention on 8 TRN2 NeuronCores.

Problem: B=2, T=4096, D=512, H=8 heads (hd=64), fp32 in/out.

Sharding: core c in 0..7 handles batch b = c//4 and head pair g = c%4
(heads 2g, 2g+1 -> D-slice [128g, 128g+128)). Each core computes
    partial_out = concat_h( softmax(causal(Q_h K_h^T / 8)) V_h ) @ W_O[slice]
for its two heads; the host sums the 4 partials per batch and adds b_O.

Pipeline design (v4). ScalarE exp() is the hard floor (~17M causal score
elements per core at 1 elem/cycle/lane @1.2GHz) and TensorE is a close
second, so everything is organised to keep both streams dense and warm
(the HAM clock gate halves the PE clock after ~3.4us of idle):

  - Score slot = ONE key block x BOTH heads in one [128,1024] PSUM tile
    (head A bank 0, head B bank 1), one exp ACTIVATE per slot through a
    [128,2,n] strided AP -> 144 calls total.
  - One GLOBAL unit stream across all q-slices: scores run 1 unit ahead
    of exp and PV runs 1 unit behind, ACROSS slice boundaries, so the
    exp stream never waits at a boundary.
  - QKV projection (slice s+1) and O-projection slots live in their OWN
    2-bank PSUM pool ("ex"), so they never steal slots from the
    exp-paced score rotation (which would stall the exp stream by one
    exp period per stolen slot).
  - Normalisation happens AFTER the O-projection: each head's O-proj is
    a separate 64-contraction matmul (row-tiled, concurrent), and the
    PSUM->SBUF evac scales by 1/L per output row (per-partition scalar)
    and fuses the two heads' add:  o = ps_A * (1/L_A) + ps_B * (1/L_B).
    1/L reaches [128,1] partition form via one DRAM-bounce spread per
    head per slice; no broadcast round-trip, no Z normalisation pass.
  - V bias is a DVE broadcast-add fused into the V PSUM->SBUF evac; each
    V t-tile owns a full PSUM bank (PE-write + DVE-read of the same bank
    is a fatal collision).
  - exp table set is preloaded with a dummy call during the DMA phase;
    input DMAs are ordered so the slice-0 projection starts ASAP.
"""

import numpy as np

import concourse.bass as bass
import concourse.mybir as mybir
from concourse.tile import TileContext
from concourse.bass_utils import run_bass_kernel_spmd

try:
    import ml_dtypes

    _BF16 = ml_dtypes.bfloat16
except ImportError:  # pragma: no cover
    _BF16 = None

F32 = mybir.dt.float32
BF16 = mybir.dt.bfloat16

B, T, D, H = 2, 4096, 512, 8
HD = D // H  # 64
SW = 512  # q-slice width
NS = T // SW  # 8 q-slices
NKC = D // 128  # 4 contraction chunks for the projections
NTT = T // 128  # 32 t-tiles / key blocks
NEG = -1.0e9


def _split_waits(nc, max_waits=1):
    """The staged walrus rejects >1 semaphore wait per instruction; hoist
    extras onto same-engine NoOps inserted right before the instruction."""
    counter = 0
    for f in nc.m.functions:
        for blk in f.blocks:
            insts = blk.instructions
            out, changed = [], False
            for ins in insts:
                si = getattr(ins, "sync_info", None)
                waits = list(si.on_wait) if si is not None and si.on_wait else []
                if len(waits) > max_waits:
                    changed = True
                    for w in waits[:-max_waits]:
                        counter += 1
                        nop = mybir.InstNoOp(
                            name=f"I-wsplit-{counter}",
                            engine=ins.engine,
                            ins=[],
                            outs=[],
                        )
                        nop.sync_info = mybir.SyncInfo(on_wait=[w], on_update=[])
                        out.append(nop)
                    ins.sync_info = mybir.SyncInfo(
                        on_wait=waits[-max_waits:], on_update=list(si.on_update)
                    )
                out.append(ins)
            if changed:
                blk.instructions = out
    return counter


def build_nc():
    nc = bass.Bass("TRN2")

    xt = nc.dram_tensor("xt", [D, T], BF16, kind="ExternalInput")
    wq = nc.dram_tensor("wq", [D, 128], BF16, kind="ExternalInput")
    wk = nc.dram_tensor("wk", [D, 128], BF16, kind="ExternalInput")
    wv = nc.dram_tensor("wv", [D, 128], BF16, kind="ExternalInput")
    wo = nc.dram_tensor("wo", [128, D], BF16, kind="ExternalInput")
    bq = nc.dram_tensor("bq", [128, 1], F32, kind="ExternalInput")
    bk = nc.dram_tensor("bk", [128, 1], F32, kind="ExternalInput")
    bv = nc.dram_tensor("bv", [1, 128], F32, kind="ExternalInput")
    out = nc.dram_tensor("out", [T, D], BF16, kind="ExternalOutput")

    # maskneg[k, q'] = 0 where q' >= k else NEG  (S^T diagonal subtile mask)
    ident_np = np.eye(128, dtype=np.float32)
    mask_np = np.where(
        np.arange(128)[None, :] >= np.arange(128)[:, None], 0.0, NEG
    ).astype(np.float32)
    ident_dram = nc.inline_tensor(ident_np.astype(_BF16), name="identc")
    mask_dram = nc.inline_tensor(mask_np.astype(_BF16), name="maskc")

    with TileContext(nc) as tc:
        with (
            tc.tile_pool(name="singles", bufs=1) as singles,
            tc.tile_pool(name="ps", bufs=2, space="PSUM") as ps,
            tc.tile_pool(name="ex", bufs=1, space="PSUM") as ext,
            tc.tile_pool(name="zps", bufs=1, space="PSUM") as zps,
            tc.tile_pool(name="pt", bufs=6) as ptp,
            tc.tile_pool(name="sl", bufs=2) as slp,
            tc.tile_pool(name="zn", bufs=3) as znp,
            tc.tile_pool(name="outp", bufs=4) as outp,
            tc.tile_pool(name="drp", bufs=2, space="DRAM") as drp,
        ):
            # ---- static SBUF; DMA order matters: slice-0 projection inputs
            # first (each weight is ONE batched DMA -- the sync queue costs
            # ~600ns of issue time per descriptor), bulk xt via the idle
            # GpSimd SWDGE queue ----
            ident_sb = singles.tile([128, 128], BF16, tag="ident")
            mask_sb = singles.tile([128, 128], BF16, tag="mask")
            nc.sync.dma_start(out=ident_sb[:, :], in_=ident_dram[:, :])
            nc.sync.dma_start(out=mask_sb[:, :], in_=mask_dram[:, :])
            xt_sb = [
                [
                    singles.tile(
                        [128, SW], BF16, tag=f"xt{c}_{s}", name=f"xt_sb{c}_{s}"
                    )
                    for s in range(NS)
                ]
                for c in range(NKC)
            ]
            for c in range(NKC):
                nc.sync.dma_start(
                    out=xt_sb[c][0][:, :], in_=xt[c * 128 : (c + 1) * 128, 0:SW]
                )
            wq_sb = singles.tile([128, NKC, 128], BF16, tag="wq")
            wk_sb = singles.tile([128, NKC, 128], BF16, tag="wk")
            wv_sb = singles.tile([128, NKC, 128], BF16, tag="wv")
            nc.sync.dma_start(
                out=wq_sb[:, :, :], in_=wq[:, :].rearrange("(c p) n -> p c n", p=128)
            )
            nc.sync.dma_start(
                out=wk_sb[:, :, :], in_=wk[:, :].rearrange("(c p) n -> p c n", p=128)
            )
            bq_sb = singles.tile([128, 1], F32, tag="bq")
            nc.sync.dma_start(out=bq_sb[:, :], in_=bq[:, :])
            # preload the exp table set while the rest of the DMAs stream in
            warm_sb = singles.tile([1, 1], BF16, tag="warm")
            nc.scalar.activation(
                out=warm_sb[:, :],
                in_=bq_sb[0:1, 0:1],
                func=mybir.ActivationFunctionType.Exp,
                scale=0.125,
            )
            bk_sb = singles.tile([128, 1], F32, tag="bk")
            nc.sync.dma_start(out=bk_sb[:, :], in_=bk[:, :])
            nc.sync.dma_start(
                out=wv_sb[:, :, :], in_=wv[:, :].rearrange("(c p) n -> p c n", p=128)
            )
            # b_V broadcast across all 128 partitions: [128, 128] f32
            bvrep_sb = singles.tile([128, 128], F32, tag="bvrep")
            bvap = bv[:, :]
            bv_src = bass.AP(
                tensor=bvap.tensor,
                offset=bvap.offset,
                ap=[[0, 128]] + list(bvap.ap[1:]),
            )
            nc.sync.dma_start(out=bvrep_sb[:, :], in_=bv_src)
            wo_sb = singles.tile([128, D], BF16, tag="wo")
            nc.sync.dma_start(out=wo_sb[:, :], in_=wo[:, :])
            for s in range(1, NS):
                for c in range(NKC):
                    nc.gpsimd.dma_start(
                        out=xt_sb[c][s][:, :],
                        in_=xt[c * 128 : (c + 1) * 128, s * SW : (s + 1) * SW],
                    )

            qt_sb = [
                singles.tile([128, SW], BF16, tag=f"qt{s}", name=f"qt_sb{s}")
                for s in range(NS)
            ]
            kt_sb = [
                singles.tile([128, SW], BF16, tag=f"kt{s}", name=f"kt_sb{s}")
                for s in range(NS)
            ]
            # V_aug pair per key block: [128(t), 130]; cols 0:64 head A,
            # col 64 ones(A), cols 65:129 head B, col 129 ones(B)
            va_sb = [
                singles.tile([128, 2 * (HD + 1)], BF16, tag=f"va{t}", name=f"va_sb{t}")
                for t in range(NTT)
            ]
            for t in range(NTT):
                nc.vector.memset(va_sb[t][:, HD : HD + 1], 1.0)
                nc.vector.memset(va_sb[t][:, 2 * HD + 1 : 2 * HD + 2], 1.0)

            hrows = (slice(0, HD), slice(HD, 128))

            # ---------- emit helpers ----------
            def emit_proj_qk(s):
                sg = ext.tile([128, 2 * SW], F32, tag="ex", name="ps_qk")
                for idx, (w_sb, b_sb, dst) in enumerate(
                    ((wq_sb, bq_sb, qt_sb[s]), (wk_sb, bk_sb, kt_sb[s]))
                ):
                    off = idx * SW
                    for c in range(NKC):
                        nc.tensor.matmul(
                            sg[:, off : off + SW],
                            lhsT=w_sb[:, c, :],
                            rhs=xt_sb[c][s][:, :],
                            start=(c == 0),
                            stop=(c == NKC - 1),
                            skip_group_check=True,
                        )
                    nc.vector.tensor_scalar_add(
                        dst[:, :], sg[:, off : off + SW], b_sb[:, :]
                    )

            def emit_proj_v_half(s, half):
                # one full PSUM bank per t-tile: a DVE evac of one t-tile may
                # run while PE still writes another, and PE-write + DVE-read
                # in the SAME bank is a fatal collision
                if True:
                    sg = ext.tile([128, 2 * SW], F32, tag="ex", name="ps_v")
                    for tt in (2 * half, 2 * half + 1):
                        t = 4 * s + tt
                        off = (tt % 2) * SW
                        for c in range(NKC):
                            nc.tensor.matmul(
                                sg[:, off : off + 128],
                                lhsT=xt_sb[c][s][:, tt * 128 : (tt + 1) * 128],
                                rhs=wv_sb[:, c, :],
                                start=(c == 0),
                                stop=(c == NKC - 1),
                                skip_group_check=True,
                            )
                        # evac + b_V add in one op: dst [128,2,64] strided
                        # (skip the ones columns), src/bias [128,2,64]
                        dst3 = va_sb[t][:, 0 : 2 * (HD + 1)].rearrange(
                            "p (a b) -> p a b", a=2
                        )[:, :, 0:HD]
                        src3 = sg[:, off : off + 128].rearrange(
                            "p (a b) -> p a b", a=2
                        )
                        bv3 = bvrep_sb[:, :].rearrange("p (a b) -> p a b", a=2)
                        nc.vector.tensor_add(dst3, src3, bv3)

            def emit_scores(unit):
                s, kb, n, qlo = unit[:4]
                qs = s * SW
                diag = kb * 128 >= qs
                sg = ps.tile([128, 2 * SW], F32, tag="sg", name="ps_sg")
                unit[4] = sg
                for h in range(2):
                    off = h * SW
                    nc.tensor.matmul(
                        sg[:, off : off + n],
                        lhsT=kt_sb[kb // 4][
                            hrows[h], (kb % 4) * 128 : (kb % 4 + 1) * 128
                        ],
                        rhs=qt_sb[s][hrows[h], qlo - qs : qlo - qs + n],
                        start=True,
                        stop=not diag,
                        skip_group_check=True,
                        tile_position=(h * HD, 0),
                    )
                if diag:
                    for h in range(2):
                        nc.tensor.matmul(
                            sg[:, h * SW : h * SW + 128],
                            lhsT=ident_sb[:, :],
                            rhs=mask_sb[:, :],
                            start=False,
                            stop=True,
                            skip_group_check=True,
                        )

            def emit_exp(unit):
                s, kb, n, qlo, sg = unit[:5]
                pt = ptp.tile([128, 2 * SW], BF16, tag="pt", name="pt")
                in3 = sg[:, :].rearrange("p (a b) -> p a b", a=2)[:, :, 0:n]
                out3 = pt[:, 0 : 2 * n].rearrange("p (a b) -> p a b", a=2)
                nc.scalar.activation(
                    out=out3,
                    in_=in3,
                    func=mybir.ActivationFunctionType.Exp,
                    scale=0.125,
                )
                unit[4] = pt

            def emit_pv(unit, zaug):
                s, kb, n, qlo, pt = unit[:5]
                qs = s * SW
                nkb = 4 * (s + 1)
                for h in range(2):
                    nc.tensor.matmul(
                        zaug[h][0 : HD + 1, qlo - qs : SW],
                        lhsT=va_sb[kb][:, h * (HD + 1) : (h + 1) * (HD + 1)],
                        rhs=pt[:, h * n : h * n + n],
                        start=(kb == 0),
                        stop=(kb == nkb - 1),
                        skip_group_check=True,
                    )

            # O-proj work items: (zpair, li_pair_or_ri_pair, qs, j, ready_at).
            # Drained one q-tile per stream iteration once ready_at passes,
            # so the 1.3us of DVE evac per q-tile never bunches up.  The
            # reciprocals are deferred to the first q-tile item (j==0): in
            # emit_zfinish they would block the DVE FIFO on the L-spread
            # DRAM round-trip.
            oproj_work = []

            def emit_zfinish(sp, zaug, g0=0, last=False):
                """Evacuate slice sp's Z/L accumulators and set up the
                deferred, post-O-proj normalisation: unnormalised Z pair
                stacked [128,512] bf16 (head B shifted to partitions
                64..127 via GpSimd) + per-q-tile [128,1] 1/L spreads."""
                zpair = znp.tile([128, SW], BF16, tag="zn")
                znb = slp.tile([HD, SW], BF16, tag="znb")
                nc.vector.tensor_copy(zpair[0:HD, :], zaug[0][0:HD, :])
                nc.vector.tensor_copy(znb[:, :], zaug[1][0:HD, :])
                lri = []
                for h in range(2):
                    lr = slp.tile([1, SW], F32, tag=f"lr{h}", name="lrow")
                    if last:
                        # ScalarE is idle at the tail; take it off DVE
                        nc.scalar.copy(lr[:, :], zaug[h][HD : HD + 1, :])
                    else:
                        nc.vector.tensor_copy(lr[:, :], zaug[h][HD : HD + 1, :])
                    rd = drp.tile([1, SW], F32, tag=f"rd{h}", name="rd")
                    nc.sync.dma_start(out=rd[:, :], in_=lr[:, :])
                    # [1,512] -> [128,4]: partition p, col j = L[j*128+p]
                    li = znp.tile([128, SW // 128], F32, tag=f"li{h}", name="li")
                    nc.sync.dma_start(
                        out=li[:, :],
                        in_=rd[0, :].rearrange("(a p) -> p a", p=128),
                    )
                    lri.append(li)
                nc.gpsimd.dma_start(out=zpair[HD:128, :], in_=znb[:, :])
                state = {"li": lri, "ri": None}
                for j in range(4):
                    oproj_work.append((zpair, state, sp * SW, j, g0 + 10 + 3 * j))

            def emit_oproj_mms(item, pool=None):
                zpair, state, qs_t, j, _ = item
                op = (pool or ext).tile(
                    [128, 2 * SW], F32,
                    tag="ex" if pool is None else "sg",
                    name="ps_o",
                )
                jq = slice(j * 128, (j + 1) * 128)
                nc.tensor.matmul(
                    op[:, 0:SW],
                    lhsT=zpair[0:HD, jq],
                    rhs=wo_sb[0:HD, :],
                    start=True,
                    stop=True,
                    skip_group_check=True,
                    tile_position=(0, 0),
                )
                nc.tensor.matmul(
                    op[:, SW : 2 * SW],
                    lhsT=zpair[HD:128, jq],
                    rhs=wo_sb[HD:128, :],
                    start=True,
                    stop=True,
                    skip_group_check=True,
                    tile_position=(HD, 0),
                )
                return op

            def emit_oproj_evac(item, op, scalar_assist=False):
                zpair, state, qs_t, j, _ = item
                if state["ri"] is None:
                    ra = znp.tile([128, SW // 128], F32, tag="ri0", name="ri")
                    nc.vector.reciprocal(ra[:, :], state["li"][0][:, :])
                    rb = znp.tile([128, SW // 128], F32, tag="ri1", name="ri")
                    nc.vector.reciprocal(rb[:, :], state["li"][1][:, :])
                    state["ri"] = (ra, rb)
                ra, rb = state["ri"]
                o_tmp = outp.tile([128, D], F32, tag="otmp", name="o_tmp")
                if scalar_assist:
                    nc.scalar.mul(o_tmp[:, :], op[:, SW : 2 * SW], rb[:, j : j + 1])
                else:
                    nc.vector.tensor_scalar_mul(
                        o_tmp[:, :], op[:, SW : 2 * SW], rb[:, j : j + 1]
                    )
                o_sb = outp.tile([128, D], BF16, tag="ot", name="o_sb")
                nc.vector.scalar_tensor_tensor(
                    o_sb[:, :],
                    op[:, 0:SW],
                    ra[:, j : j + 1],
                    o_tmp[:, :],
                    op0=mybir.AluOpType.mult,
                    op1=mybir.AluOpType.add,
                )
                r0 = qs_t + j * 128
                nc.sync.dma_start(out=out[r0 : r0 + 128, :], in_=o_sb[:, :])

            # HAM warm-up: the PE clock gate needs ~3.4us of sustained
            # activity to reach 2.4GHz; burn the DMA-wait on dummy matmuls
            # (dep only on the tiny ident/mask DMAs, which are issued first)
            # so slice 0 runs warm and the real work is never behind them.
            warm_ps = ext.tile([128, 2 * SW], F32, tag="ex", name="ps_warm")
            for _ in range(32):
                nc.tensor.matmul(
                    warm_ps[:, 0:128],
                    lhsT=ident_sb[:, :],
                    rhs=mask_sb[:, :],
                    start=True,
                    stop=True,
                    skip_group_check=True,
                )

            # ---------- global unit stream ----------
            stream = []
            first_of_slice = {}
            for s in range(NS):
                qs = s * SW
                first_of_slice[s] = len(stream)
                for kb in range(4 * (s + 1)):
                    qlo = max(qs, kb * 128)
                    stream.append([s, kb, qs + SW - qlo, qlo, None])
            G = len(stream)

            def emit_oproj_qtile(item, scalar_assist=False, pool=None):
                op = emit_oproj_mms(item, pool=pool)
                emit_oproj_evac(item, op, scalar_assist=scalar_assist)

            def inserts(s, i, g):
                # extras schedule (one PSUM "ex" slot each, at most ~1/iter):
                # proj for slice s+1 must be fully emitted before the scores
                # of s+1 u0 (iter L-1); V halves feed PV with plenty of lag
                if s == 0:
                    if i == 0:
                        emit_proj_v_half(0, 0)
                    elif i == 1:
                        emit_proj_v_half(0, 1)
                    elif i == 2:
                        emit_proj_qk(1)
                    elif i == 3:
                        emit_proj_v_half(1, 0)
                elif s == 1:
                    if i == 0:
                        emit_proj_v_half(1, 1)
                    elif i == 5:
                        emit_proj_qk(2)
                    elif i == 6:
                        emit_proj_v_half(2, 0)
                    elif i == 7:
                        emit_proj_v_half(2, 1)
                elif s + 1 < NS:
                    if i == 5:
                        emit_proj_qk(s + 1)
                    elif i == 6:
                        emit_proj_v_half(s + 1, 0)
                    elif i == 7:
                        emit_proj_v_half(s + 1, 1)
                if oproj_work and g >= oproj_work[0][4]:
                    emit_oproj_qtile(oproj_work.pop(0))

            emit_proj_qk(0)
            emit_scores(stream[0])
            zaug = None
            prev_zaug = None
            for g in range(G):
                s, kb = stream[g][0], stream[g][1]
                emit_exp(stream[g])
                if g + 1 < G:
                    emit_scores(stream[g + 1])
                if g >= 1:
                    ps_, pkb = stream[g - 1][0], stream[g - 1][1]
                    if pkb == 0:
                        # stream[g-1] opens slice ps_: finish the previous
                        # slice's accumulators, then claim fresh ones
                        if prev_zaug is not None:
                            emit_zfinish(ps_ - 1, prev_zaug, g0=g)
                        zaug = [
                            zps.tile([HD + 1, SW], F32, tag="za", name="zauga"),
                            zps.tile([HD + 1, SW], F32, tag="zb", name="zaugb"),
                        ]
                        prev_zaug = zaug
                    emit_pv(stream[g - 1], zaug)
                inserts(s, g - first_of_slice[s], g)
            emit_pv(stream[G - 1], zaug)

            # tail: last slice's Z finish + remaining O-projections
            emit_zfinish(NS - 1, zaug, g0=G, last=True)
            # issue all four O-proj matmul pairs up front (they only need
            # zpair; PSUM results wait in ps/ext buffers) so they overlap
            # the 1/L spread DMAs; then pipeline the scaled evacs
            tail_items = []
            tail_pools = [ps, ps, None, ps]
            while oproj_work:
                item = oproj_work.pop(0)
                pool = tail_pools[len(tail_items) % 4]
                tail_items.append((item, emit_oproj_mms(item, pool=pool)))
            for item, op in tail_items:
                emit_oproj_evac(item, op, scalar_assist=True)

    _split_waits(nc)
    return nc


_NC_CACHE = {}


def _get_nc():
    if "nc" not in _NC_CACHE:
        _NC_CACHE["nc"] = build_nc()
    return _NC_CACHE["nc"]


def make_in_maps(combined_embed, W_K, b_K, W_Q, b_Q, W_V, b_V, W_O, b_O):
    f32 = np.float32
    in_maps = []
    for c in range(8):
        b = c // 4
        g = c % 4
        sl = slice(g * 128, (g + 1) * 128)
        xt = np.ascontiguousarray(np.asarray(combined_embed[b], f32).T)
        in_maps.append(
            {
                "xt": xt.astype(_BF16),
                "wq": np.ascontiguousarray(np.asarray(W_Q, f32)[:, sl]).astype(_BF16),
                "wk": np.ascontiguousarray(np.asarray(W_K, f32)[:, sl]).astype(_BF16),
                "wv": np.ascontiguousarray(np.asarray(W_V, f32)[:, sl]).astype(_BF16),
                "wo": np.ascontiguousarray(np.asarray(W_O, f32)[sl, :]).astype(_BF16),
                "bq": np.asarray(b_Q, f32)[sl].reshape(128, 1).copy(),
                "bk": np.asarray(b_K, f32)[sl].reshape(128, 1).copy(),
                "bv": np.asarray(b_V, f32)[sl].reshape(1, 128).copy(),
            }
        )
    return in_maps


def run_cores(in_maps, **kwargs):
    nc = _get_nc()
    return run_bass_kernel_spmd(nc, in_maps, core_ids=list(range(8)), **kwargs)


def kernel(
    combined_embed, W_K, b_K, W_Q, b_Q, W_V, b_V, W_O, b_O
):  # full inputs -> full output
    in_maps = make_in_maps(
        combined_embed, W_K, b_K, W_Q, b_Q, W_V, b_V, W_O, b_O
    )
    res = run_cores(in_maps)
    out = np.zeros((B, T, D), np.float32)
    for c in range(8):
        out[c // 4] += np.asarray(res.results[c]["out"], np.float32)
    out += np.asarray(b_O, np.float32)[None, None, :]
    return out
